# revision 13
# baseline (speedup 1.0000x reference)
"""BitLinearOptimized Trainium2 kernel — 8-core SPMD, self-contained.

kernel(**inputs) takes the FULL inputs (input [8192,4096] f32,
weight [4096,4096] f32 ternary, weight_scale [1] f32, bias [4096] f32)
and returns the FULL output [8192, 4096] f32.

Sharding: input row-sharded 8 ways (each core quantizes its rows),
weight sharded along out_features (each core group-sums its shard, then
AllGather of the small reduced w_sumT so every core holds all out
features). A global absmax AllReduce(max) provides act_scale. Each core
computes outT[:, its rows] = w_sumT.T @ x_sumT (f16 operands, fp32
PSUM — exact integer arithmetic), applies scale+bias, host concatenates.

v3 layout/overlap rework:
- x loads issued first on all rings; absmax -> AllReduce triggers ~55us.
- w loads on the SWDGE ring; w group-sum via single tensor_reduce ops;
  AllGather triggered independently of (and concurrent with) AllReduce.
- x stays resident in SBUF and is quantized IN PLACE (no re-read).
- wsT stored partition-major so the AllGather result is loaded with
  128 x 8KB contiguous descriptors per remote block (was 1024 x 256B).
- group-sums in f16 (values are small integers -> exact).
"""

import numpy as np

import concourse.bass as bass
from concourse import bacc
import concourse.mybir as mybir
import concourse.tile as tile

F32 = mybir.dt.float32
F16 = mybir.dt.float16
MAGIC_C = float(np.float32(1.5 * 2**23))

# problem shape (hardcoded per contest contract)
N_FULL, IN_F, OUT_F, NCORES = 8192, 4096, 4096, 8


def build_bitlinear(N=N_FULL, IN=IN_F, OUT=OUT_F, ncores=NCORES):
    P = 128
    ROWS = N // ncores          # rows per core (1024)
    OCOLS = OUT // ncores       # out features per core (512)
    G = IN // 4                 # groups (1024)
    RT = ROWS // P              # row tiles (8)
    GT = G // P                 # k tiles for matmul (8)
    WT = OCOLS // P             # w shard row tiles (4)
    NCH = 512                   # matmul moving free dim
    NNT = ROWS // NCH           # row chunks (2)
    WCH = 1024                  # w load chunk (free dim)
    WCT = IN // WCH             # w chunks per tile (4)
    HC = IN // 2                # quantize half-tile (2048)
    assert ROWS % P == 0 and G % P == 0 and OCOLS % P == 0

    core_ids = list(range(ncores))
    OBT = OUT // P              # output o blocks (32)
    nc = bacc.Bacc(num_devices=ncores)

    x_d = nc.declare_dram_parameter("x_loc", [ROWS, IN], F32, isOutput=False)
    w_d = nc.declare_dram_parameter("w_loc", [OCOLS, IN], F32, isOutput=False)
    ws_d = nc.declare_dram_parameter("wscale", [1, 1], F32, isOutput=False)
    bias_d = nc.declare_dram_parameter("bias", [OUT], F32, isOutput=False)
    outT_d = nc.declare_dram_parameter("outT", [OUT, ROWS], F32, isOutput=True)

    # collective bounce buffers (internal DRAM; outputs Shared)
    ar_in_d = nc.dram_tensor("ar_in", [128], F32)
    ar_out_d = nc.dram_tensor("ar_out", [128], F32, addr_space="Shared")
    scal_d = nc.dram_tensor("scal_bounce", [8], F32)
    # partition-major reduced weights: [p, wt, k, o] so a remote block is
    # 8KB contiguous per partition.
    wsT_loc_d = nc.dram_tensor("wsT_loc", [P, WT * GT * P], F16)
    wsT_all_d = nc.dram_tensor("wsT_all", [ncores * P, WT * GT * P], F16,
                               addr_space="Shared")

    with tile.TileContext(nc) as tc:
        with (
            tc.tile_pool(name="xp", bufs=RT) as xp,           # resident x
            tc.tile_pool(name="qp", bufs=3) as qp,             # q half-tiles
            tc.tile_pool(name="xsum", bufs=2) as xsump,        # xs [P, G]
            tc.tile_pool(name="wld", bufs=2) as wldp,          # w chunks
            tc.tile_pool(name="wsum", bufs=2) as wsump,
            tc.tile_pool(name="w3T", bufs=2) as w3Tp,          # w transposed
            tc.tile_pool(name="xsT", bufs=1) as xsTp,
            tc.tile_pool(name="wst", bufs=2) as wstp,          # remote stationary
            tc.tile_pool(name="outp", bufs=3) as outp,
            tc.tile_pool(name="cst", bufs=1) as cst,
            tc.tile_pool(name="ps", bufs=7, space="PSUM") as psp,
        ):
            # ---------------- phase A: x loads (both HWDGE rings) ------------
            xta = []
            for rt in range(RT):
                xt = xp.tile([P, IN], F32, tag="xp", name=f"x{rt}")
                eng = nc.sync if rt % 2 == 0 else nc.scalar
                eng.dma_start(out=xt[:], in_=x_d[rt * P:(rt + 1) * P, :])
                xta.append(xt)

            # w loads right behind x on the SWDGE ring
            wla = []
            for wt in range(WT):
                for ck in range(WCT):
                    wl = wldp.tile([P, WCH], F32, tag="wld")
                    nc.gpsimd.dma_start(
                        out=wl[:], in_=w_d[wt * P:(wt + 1) * P,
                                           ck * WCH:(ck + 1) * WCH])
                    wla.append(wl)

            mxcol = cst.tile([P, RT], F32, tag="mxcol")
            for rt in range(RT):
                nc.vector.tensor_reduce(out=mxcol[:, rt:rt + 1], in_=xta[rt][:],
                                        axis=mybir.AxisListType.X,
                                        op=mybir.AluOpType.max,
                                        apply_absolute_value=True)
            mx1 = cst.tile([P, 1], F32, tag="mx1")
            nc.vector.tensor_reduce(out=mx1[:], in_=mxcol[:],
                                    axis=mybir.AxisListType.X,
                                    op=mybir.AluOpType.max)
            # AllReduce(max) of the [128] per-partition max vector
            nc.sync.dma_start(out=ar_in_d[:].rearrange("(p s) -> p s", p=P),
                              in_=mx1[:])
            nc.gpsimd.collective_compute(
                "AllReduce", mybir.AluOpType.max,
                replica_groups=[core_ids],
                ins=[ar_in_d[:]], outs=[ar_out_d[:]],
            )

            # ---------------- w path: group-sum + transpose -----------------
            with nc.allow_low_precision(reason="w_sum in [-4,4], exact in f16"):
                wsums = []
                for wt in range(WT):
                    wsum_t = wsump.tile([P, G], F16, tag="wsum")
                    for ck in range(WCT):
                        gch = WCH // 4
                        nc.vector.tensor_reduce(
                            out=wsum_t[:, ck * gch:(ck + 1) * gch],
                            in_=wla[wt * WCT + ck][:]
                                .rearrange("p (g f) -> p g f", f=4),
                            axis=mybir.AxisListType.X,
                            op=mybir.AluOpType.add)
                    wsums.append(wsum_t)
            for wt in range(WT):
                w3T = w3Tp.tile([P, GT, P], F16, tag="w3T", name=f"w3T{wt}")
                nc.scalar.dma_start_transpose(w3T[:], wsums[wt][:])
                nc.sync.dma_start(
                    out=wsT_loc_d[:, wt * GT * P:(wt + 1) * GT * P]
                        .rearrange("p (a o) -> p a o", a=GT),
                    in_=w3T[:])
            nc.gpsimd.collective_compute(
                "AllGather", mybir.AluOpType.bypass,
                replica_groups=[core_ids],
                ins=[wsT_loc_d[:]], outs=[wsT_all_d[:]],
            )

            # ---------------- post-AllReduce scalars ------------------------
            gmax = cst.tile([1, P], F32, tag="gmax")
            nc.sync.dma_start(out=gmax[:],
                              in_=ar_out_d[:].rearrange("(a b) -> a b", a=1))
            mloc = cst.tile([1, 1], F32, tag="mloc")
            nc.vector.tensor_reduce(out=mloc[:], in_=gmax[:],
                                    axis=mybir.AxisListType.X,
                                    op=mybir.AluOpType.max)
            # act_scale = gmax/127; recip = 1/act_scale; sc = ws*act_scale/4
            asc = cst.tile([1, 1], F32, tag="asc")
            nc.vector.tensor_scalar(out=asc[:], in0=mloc[0:1, 0:1],
                                    scalar1=float(np.float32(1.0 / 127.0)),
                                    scalar2=None,
                                    op0=mybir.AluOpType.mult)
            recip = cst.tile([1, 1], F32, tag="recip")
            nc.vector.reciprocal(out=recip[:], in_=asc[:])
            ws_sb = cst.tile([1, 1], F32, tag="ws_sb")
            nc.scalar.dma_start(out=ws_sb[:], in_=ws_d[:])
            sc = cst.tile([1, 1], F32, tag="sc")
            nc.vector.tensor_tensor(out=sc[:], in0=ws_sb[:], in1=asc[:],
                                    op=mybir.AluOpType.mult)
            nc.vector.tensor_scalar(out=sc[:], in0=sc[:], scalar1=0.25,
                                    scalar2=None, op0=mybir.AluOpType.mult)
            sc2 = cst.tile([1, 2], F32, tag="sc2")
            nc.vector.tensor_copy(out=sc2[0:1, 0:1], in_=recip[:])
            nc.vector.tensor_copy(out=sc2[0:1, 1:2], in_=sc[:])
            nc.sync.dma_start(out=scal_d[0:2].rearrange("(a b) -> a b", a=1),
                              in_=sc2[:])
            scbc = cst.tile([P, 2], F32, tag="scbc")
            nc.sync.dma_start(out=scbc[:],
                              in_=bass.AP(scal_d, 0, [[0, P], [1, 2]]))
            recip_bc = scbc[:, 0:1]
            sc_bc = scbc[:, 1:2]

            # bias in [P, OUT/P] layout: bias_sb[p, b] = bias[b*128+p]
            bias_sb = cst.tile([P, OUT // P], F32, tag="bias_sb")
            nc.sync.dma_start(out=bias_sb[:],
                              in_=bias_d[:].rearrange("(b p) -> p b", p=P))

            # ---------------- quantize (in place) + group-sum + transpose ---
            # pass1 (DVE): x <- x*recip + MAGIC   (rounds to int in mantissa)
            # pass2 (Act/GpSimd alternate): q = (tq - MAGIC) as f16, halves
            # reduce (DVE): xs[g] = sum of q group-of-4 (f16 exact, <=508)
            # transpose (DVE ring): xs [P,G] -> xsT3[:, :, rt*P:...]
            xsT3 = xsTp.tile([P, GT, ROWS], F16, tag="xsT3")
            with nc.allow_low_precision(reason="x_q sums <=508, exact in f16"):
                for rt in range(RT):
                    xt = xta[rt]
                    nc.vector.tensor_scalar(out=xt[:], in0=xt[:],
                                            scalar1=recip_bc, scalar2=MAGIC_C,
                                            op0=mybir.AluOpType.mult,
                                            op1=mybir.AluOpType.add)
                    p2eng = nc.scalar if rt % 2 == 0 else nc.gpsimd
                    xs = xsump.tile([P, G], F16, tag="xsum")
                    for h in range(2):
                        qh = qp.tile([P, HC], F16, tag="qp")
                        if rt % 2 == 0:
                            p2eng.activation(
                                out=qh[:], in_=xt[:, h * HC:(h + 1) * HC],
                                func=mybir.ActivationFunctionType.Copy,
                                bias=-MAGIC_C, scale=1.0)
                        else:
                            p2eng.tensor_scalar(
                                out=qh[:], in0=xt[:, h * HC:(h + 1) * HC],
                                scalar1=-MAGIC_C, scalar2=None,
                                op0=mybir.AluOpType.add)
                        nc.vector.tensor_reduce(
                            out=xs[:, h * (G // 2):(h + 1) * (G // 2)],
                            in_=qh[:].rearrange("p (g f) -> p g f", f=4),
                            axis=mybir.AxisListType.X,
                            op=mybir.AluOpType.add)
                    # transposes: odd early tiles on sync (before the wst
                    # loads), the rest on scalar between pass2's
                    treng = nc.sync if rt in (1, 3) else nc.scalar
                    treng.dma_start_transpose(
                        xsT3[:, :, rt * P:(rt + 1) * P], xs[:])

            # ---------------- matmul + epilogue ------------------------------
            # nn outer; each remote stationary block is one contiguous
            # 128 x 8KB load from the gathered buffer.
            for nn in range(NNT):
                for rblk in range(ncores):
                    wst = wstp.tile([P, WT * GT * P], F16, tag="wst",
                                    name=f"wst{nn}_{rblk}")
                    nc.sync.dma_start(
                        out=wst[:],
                        in_=wsT_all_d[rblk * P:(rblk + 1) * P, :])
                    w4 = wst[:].rearrange("p (w a o) -> p w a o", w=WT, a=GT)
                    for wt in range(WT):
                        ps = psp.tile([P, NCH], F32, tag="ps",
                                      name=f"ps{nn}_{rblk}_{wt}")
                        for k in range(GT):
                            nc.tensor.matmul(
                                ps[:],
                                lhsT=w4[:, wt, k, :],
                                rhs=xsT3[:, k, nn * NCH:(nn + 1) * NCH],
                                start=(k == 0), stop=(k == GT - 1))
                        ob = rblk * WT + wt
                        ot = outp.tile([P, NCH], F32, tag="ot")
                        nc.scalar.activation(
                            out=ot[:], in_=ps[:],
                            func=mybir.ActivationFunctionType.Identity,
                            scale=sc_bc,
                            bias=bias_sb[:, ob:ob + 1])
                        eng = nc.sync if wt % 2 == 0 else nc.scalar
                        eng.dma_start(
                            out=outT_d[ob * P:(ob + 1) * P,
                                       nn * NCH:(nn + 1) * NCH],
                            in_=ot[:])

    return nc


def make_in_maps(inputs, ncores=NCORES):
    x = np.ascontiguousarray(np.asarray(inputs["input"], dtype=np.float32))
    w = np.ascontiguousarray(np.asarray(inputs["weight"], dtype=np.float32))
    ws = np.asarray(inputs["weight_scale"], dtype=np.float32).reshape(1, 1)
    b = np.ascontiguousarray(np.asarray(inputs["bias"], dtype=np.float32))
    N = x.shape[0]
    OUT = w.shape[0]
    ROWS = N // ncores
    OCOLS = OUT // ncores
    return [
        {
            "x_loc": x[c * ROWS:(c + 1) * ROWS],
            "w_loc": w[c * OCOLS:(c + 1) * OCOLS],
            "wscale": ws,
            "bias": b,
        }
        for c in range(ncores)
    ]


def assemble_output(results):
    return np.ascontiguousarray(
        np.concatenate([np.asarray(r["outT"]).T for r in results], axis=0))


_NC_CACHE = {}


def _get_nc():
    key = (N_FULL, IN_F, OUT_F, NCORES)
    if key not in _NC_CACHE:
        nc = build_bitlinear(*key)
        if not nc.is_finalized():
            nc.finalize()
        _NC_CACHE[key] = nc
    return _NC_CACHE[key]


def run_on_hw(inputs, trace=False):
    from concourse.bass_utils import run_bass_kernel_spmd
    nc = _get_nc()
    in_maps = make_in_maps(inputs)
    res = run_bass_kernel_spmd(nc, in_maps, list(range(NCORES)), trace=trace)
    return assemble_output(res.results), res


def kernel(**inputs) -> np.ndarray:
    out, _ = run_on_hw(inputs, trace=False)
    return out


# revision 14
# speedup vs baseline: 1.5379x; 1.5379x over previous
"""BitLinearOptimized Trainium2 kernel — 8-core SPMD, self-contained.

kernel(**inputs) takes the FULL inputs (input [8192,4096] f32,
weight [4096,4096] f32 ternary, weight_scale [1] f32, bias [4096] f32)
and returns the FULL output [8192, 4096] f32.

Sharding: input row-sharded 8 ways (each core quantizes its rows),
weight sharded along out_features (each core group-sums its shard, then
AllGathers the small reduced w_sumT so every core holds all out
features). A global absmax AllReduce(max) provides act_scale. Each core
computes outT[:, its rows] = w_sumT.T @ x_sumT (f16 operands, fp32
PSUM — exact integer arithmetic), applies scale+bias, host concatenates.

v4: x loads spread over all three DMA rings; x resident in SBUF and
quantized in place (no re-read); w group-sums as single tensor_reduce
ops with a deep-enough load pool to decouple DMA from DVE; the reduced
weight AllGather is split in two halves so the first half lands before
the matmul needs remote blocks; partition-major wsT layout makes every
stationary load contiguous per partition; gpsimd queue carries only
DMAs/triggers (its Q7 ALU is ~15x slower than modeled for bulk work).
"""

import numpy as np

import concourse.bass as bass
from concourse import bacc
import concourse.mybir as mybir
import concourse.tile as tile

F32 = mybir.dt.float32
F16 = mybir.dt.float16
MAGIC_C = float(np.float32(1.5 * 2**23))

# problem shape (hardcoded per contest contract)
N_FULL, IN_F, OUT_F, NCORES = 8192, 4096, 4096, 8


def build_bitlinear(N=N_FULL, IN=IN_F, OUT=OUT_F, ncores=NCORES):
    P = 128
    ROWS = N // ncores          # rows per core (1024)
    OCOLS = OUT // ncores       # out features per core (512)
    G = IN // 4                 # groups (1024)
    RT = ROWS // P              # row tiles (8)
    GT = G // P                 # k tiles for matmul (8)
    WT = OCOLS // P             # w shard row tiles (4)
    NCH = 512                   # matmul moving free dim
    NNT = ROWS // NCH           # row chunks (2)
    WCH = 1024                  # w load chunk (free dim)
    WCT = IN // WCH             # w chunks per w row tile (4)
    HC = IN // 2                # quantize half-tile (2048)
    HWT = WT // 2               # w tiles per AllGather half (2)
    assert ROWS % P == 0 and G % P == 0 and OCOLS % P == 0

    core_ids = list(range(ncores))
    nc = bacc.Bacc(num_devices=ncores)

    x_d = nc.declare_dram_parameter("x_loc", [ROWS, IN], F32, isOutput=False)
    w_d = nc.declare_dram_parameter("w_loc", [OCOLS, IN], F32, isOutput=False)
    ws_d = nc.declare_dram_parameter("wscale", [1, 1], F32, isOutput=False)
    bias_d = nc.declare_dram_parameter("bias", [OUT], F32, isOutput=False)
    outT_d = nc.declare_dram_parameter("outT", [OUT, ROWS], F32, isOutput=True)

    # collective bounce buffers (internal DRAM; outputs Shared)
    ar_in_d = nc.dram_tensor("ar_in", [128], F32)
    ar_out_d = nc.dram_tensor("ar_out", [128], F32, addr_space="Shared")
    scal_d = nc.dram_tensor("scal_bounce", [8], F32)
    # partition-major reduced weights, split in two halves (wt 0-1, wt 2-3)
    # so the first AllGather can land early. A remote block is contiguous
    # per partition.
    wsT_loc = [nc.dram_tensor(f"wsT_loc{h}", [P, HWT * GT * P], F16)
               for h in range(2)]
    wsT_all = [nc.dram_tensor(f"wsT_all{h}", [ncores * P, HWT * GT * P], F16,
                              addr_space="Shared")
               for h in range(2)]

    with tile.TileContext(nc) as tc:
        with (
            tc.tile_pool(name="xp", bufs=RT) as xp,           # resident x
            tc.tile_pool(name="qp", bufs=3) as qp,             # q half-tiles
            tc.tile_pool(name="xsum", bufs=2) as xsump,        # xs [P, G]
            tc.tile_pool(name="wld", bufs=4) as wldp,          # w chunks
            tc.tile_pool(name="wsum", bufs=2) as wsump,
            tc.tile_pool(name="w3T", bufs=2) as w3Tp,          # w transposed
            tc.tile_pool(name="xsT", bufs=1) as xsTp,
            tc.tile_pool(name="wst", bufs=3) as wstp,          # stationary
            tc.tile_pool(name="outp", bufs=4) as outp,
            tc.tile_pool(name="cst", bufs=1) as cst,
            tc.tile_pool(name="ps", bufs=7, space="PSUM") as psp,
        ):
            # ---------------- phase A: x loads on all three rings ------------
            x_eng = [nc.sync, nc.scalar, nc.gpsimd]
            xta = []
            for rt in range(RT):
                xt = xp.tile([P, IN], F32, tag="xp", name=f"x{rt}")
                x_eng[rt % 3].dma_start(out=xt[:], in_=x_d[rt * P:(rt + 1) * P, :])
                xta.append(xt)

            # w loads right behind x on the SWDGE ring
            wla = []
            for wt in range(WT):
                for ck in range(WCT):
                    wl = wldp.tile([P, WCH], F32, tag="wld")
                    nc.gpsimd.dma_start(
                        out=wl[:], in_=w_d[wt * P:(wt + 1) * P,
                                           ck * WCH:(ck + 1) * WCH])
                    wla.append(wl)

            # local absmax -> [128] vector -> AllReduce(max)
            mxcol = cst.tile([P, RT], F32, tag="mxcol")
            for rt in range(RT):
                nc.vector.tensor_reduce(out=mxcol[:, rt:rt + 1], in_=xta[rt][:],
                                        axis=mybir.AxisListType.X,
                                        op=mybir.AluOpType.max,
                                        apply_absolute_value=True)
            mx1 = cst.tile([P, 1], F32, tag="mx1")
            nc.vector.tensor_reduce(out=mx1[:], in_=mxcol[:],
                                    axis=mybir.AxisListType.X,
                                    op=mybir.AluOpType.max)
            nc.gpsimd.dma_start(out=ar_in_d[:].rearrange("(p s) -> p s", p=P),
                                in_=mx1[:])
            nc.gpsimd.collective_compute(
                "AllReduce", mybir.AluOpType.max,
                replica_groups=[core_ids],
                ins=[ar_in_d[:]], outs=[ar_out_d[:]],
            )

            # ---------------- w path: group-sum + transpose + 2 AllGathers ---
            with nc.allow_low_precision(reason="w_sum in [-4,4], exact in f16"):
                wsums = []
                for wt in range(WT):
                    wsum_t = wsump.tile([P, G], F16, tag="wsum")
                    for ck in range(WCT):
                        gch = WCH // 4
                        nc.vector.tensor_reduce(
                            out=wsum_t[:, ck * gch:(ck + 1) * gch],
                            in_=wla[wt * WCT + ck][:]
                                .rearrange("p (g f) -> p g f", f=4),
                            axis=mybir.AxisListType.X,
                            op=mybir.AluOpType.add)
                    wsums.append(wsum_t)
            for wt in range(WT):
                w3T = w3Tp.tile([P, GT, P], F16, tag="w3T", name=f"w3T{wt}")
                nc.scalar.dma_start_transpose(w3T[:], wsums[wt][:])
                h, wi = wt // HWT, wt % HWT
                nc.sync.dma_start(
                    out=wsT_loc[h][:, wi * GT * P:(wi + 1) * GT * P]
                        .rearrange("p (a o) -> p a o", a=GT),
                    in_=w3T[:])
                if wi == HWT - 1:
                    nc.gpsimd.collective_compute(
                        "AllGather", mybir.AluOpType.bypass,
                        replica_groups=[core_ids],
                        ins=[wsT_loc[h][:]], outs=[wsT_all[h][:]],
                    )

            # ---------------- post-AllReduce scalars ------------------------
            gmax = cst.tile([1, P], F32, tag="gmax")
            nc.gpsimd.dma_start(out=gmax[:],
                                in_=ar_out_d[:].rearrange("(a b) -> a b", a=1))
            mloc = cst.tile([1, 1], F32, tag="mloc")
            nc.vector.tensor_reduce(out=mloc[:], in_=gmax[:],
                                    axis=mybir.AxisListType.X,
                                    op=mybir.AluOpType.max)
            # act_scale = gmax/127; recip = 1/act_scale; sc = ws*act_scale/4
            asc = cst.tile([1, 1], F32, tag="asc")
            nc.vector.tensor_scalar(out=asc[:], in0=mloc[0:1, 0:1],
                                    scalar1=float(np.float32(1.0 / 127.0)),
                                    scalar2=None,
                                    op0=mybir.AluOpType.mult)
            recip = cst.tile([1, 1], F32, tag="recip")
            nc.vector.reciprocal(out=recip[:], in_=asc[:])
            ws_sb = cst.tile([1, 1], F32, tag="ws_sb")
            nc.scalar.dma_start(out=ws_sb[:], in_=ws_d[:])
            sc = cst.tile([1, 1], F32, tag="sc")
            nc.vector.tensor_tensor(out=sc[:], in0=ws_sb[:], in1=asc[:],
                                    op=mybir.AluOpType.mult)
            nc.vector.tensor_scalar(out=sc[:], in0=sc[:], scalar1=0.25,
                                    scalar2=None, op0=mybir.AluOpType.mult)
            sc2 = cst.tile([1, 2], F32, tag="sc2")
            nc.vector.tensor_copy(out=sc2[0:1, 0:1], in_=recip[:])
            nc.vector.tensor_copy(out=sc2[0:1, 1:2], in_=sc[:])
            nc.gpsimd.dma_start(out=scal_d[0:2].rearrange("(a b) -> a b", a=1),
                                in_=sc2[:])
            scbc = cst.tile([P, 2], F32, tag="scbc")
            nc.gpsimd.dma_start(out=scbc[:],
                                in_=bass.AP(scal_d, 0, [[0, P], [1, 2]]))
            recip_bc = scbc[:, 0:1]
            sc_bc = scbc[:, 1:2]

            # bias in [P, OUT/P] layout: bias_sb[p, b] = bias[b*128+p]
            bias_sb = cst.tile([P, OUT // P], F32, tag="bias_sb")
            nc.sync.dma_start(out=bias_sb[:],
                              in_=bias_d[:].rearrange("(b p) -> p b", p=P))

            # ---------------- quantize (in place) + group-sum + transpose ---
            xsT3 = xsTp.tile([P, GT, ROWS], F16, tag="xsT3")
            with nc.allow_low_precision(reason="x_q sums <=508, exact in f16"):
                for rt in range(RT):
                    xt = xta[rt]
                    nc.vector.tensor_scalar(out=xt[:], in0=xt[:],
                                            scalar1=recip_bc, scalar2=MAGIC_C,
                                            op0=mybir.AluOpType.mult,
                                            op1=mybir.AluOpType.add)
                    xs = xsump.tile([P, G], F16, tag="xsum")
                    for h in range(2):
                        qh = qp.tile([P, HC], F16, tag="qp")
                        nc.scalar.activation(
                            out=qh[:], in_=xt[:, h * HC:(h + 1) * HC],
                            func=mybir.ActivationFunctionType.Copy,
                            bias=-MAGIC_C, scale=1.0)
                        nc.vector.tensor_reduce(
                            out=xs[:, h * (G // 2):(h + 1) * (G // 2)],
                            in_=qh[:].rearrange("p (g f) -> p g f", f=4),
                            axis=mybir.AxisListType.X,
                            op=mybir.AluOpType.add)
                    treng = nc.sync if rt in (1, 3, 6, 7) else nc.scalar
                    treng.dma_start_transpose(
                        xsT3[:, :, rt * P:(rt + 1) * P], xs[:])

            # ---------------- matmul + epilogue ------------------------------
            # nn outer, then AllGather half, then rblk; stationary loads are
            # contiguous per partition.
            for nn in range(NNT):
                for h in range(2):
                    for rblk in range(ncores):
                        wst = wstp.tile([P, HWT * GT * P], F16, tag="wst",
                                        name=f"wst{nn}_{h}_{rblk}")
                        nc.sync.dma_start(
                            out=wst[:],
                            in_=wsT_all[h][rblk * P:(rblk + 1) * P, :])
                        w4 = wst[:].rearrange("p (w a o) -> p w a o",
                                              w=HWT, a=GT)
                        for wi in range(HWT):
                            wt = h * HWT + wi
                            ps = psp.tile([P, NCH], F32, tag="ps",
                                          name=f"ps{nn}_{h}_{rblk}_{wi}")
                            for k in range(GT):
                                nc.tensor.matmul(
                                    ps[:],
                                    lhsT=w4[:, wi, k, :],
                                    rhs=xsT3[:, k, nn * NCH:(nn + 1) * NCH],
                                    start=(k == 0), stop=(k == GT - 1))
                            ob = rblk * WT + wt
                            ot = outp.tile([P, NCH], F32, tag="ot")
                            if (rblk + wi) % 2 == 0:
                                nc.scalar.activation(
                                    out=ot[:], in_=ps[:],
                                    func=mybir.ActivationFunctionType.Identity,
                                    scale=sc_bc,
                                    bias=bias_sb[:, ob:ob + 1])
                            else:
                                nc.vector.tensor_scalar(
                                    out=ot[:], in0=ps[:],
                                    scalar1=sc_bc,
                                    scalar2=bias_sb[:, ob:ob + 1],
                                    op0=mybir.AluOpType.mult,
                                    op1=mybir.AluOpType.add)
                            nc.sync.dma_start(
                                out=outT_d[ob * P:(ob + 1) * P,
                                           nn * NCH:(nn + 1) * NCH],
                                in_=ot[:])

    return nc


def make_in_maps(inputs, ncores=NCORES):
    x = np.ascontiguousarray(np.asarray(inputs["input"], dtype=np.float32))
    w = np.ascontiguousarray(np.asarray(inputs["weight"], dtype=np.float32))
    ws = np.asarray(inputs["weight_scale"], dtype=np.float32).reshape(1, 1)
    b = np.ascontiguousarray(np.asarray(inputs["bias"], dtype=np.float32))
    N = x.shape[0]
    OUT = w.shape[0]
    ROWS = N // ncores
    OCOLS = OUT // ncores
    return [
        {
            "x_loc": x[c * ROWS:(c + 1) * ROWS],
            "w_loc": w[c * OCOLS:(c + 1) * OCOLS],
            "wscale": ws,
            "bias": b,
        }
        for c in range(ncores)
    ]


def assemble_output(results):
    return np.ascontiguousarray(
        np.concatenate([np.asarray(r["outT"]).T for r in results], axis=0))


_NC_CACHE = {}


def _get_nc():
    key = (N_FULL, IN_F, OUT_F, NCORES)
    if key not in _NC_CACHE:
        nc = build_bitlinear(*key)
        if not nc.is_finalized():
            nc.finalize()
        _NC_CACHE[key] = nc
    return _NC_CACHE[key]


def run_on_hw(inputs, trace=False):
    from concourse.bass_utils import run_bass_kernel_spmd
    nc = _get_nc()
    in_maps = make_in_maps(inputs)
    res = run_bass_kernel_spmd(nc, in_maps, list(range(NCORES)), trace=trace)
    return assemble_output(res.results), res


def kernel(**inputs) -> np.ndarray:
    out, _ = run_on_hw(inputs, trace=False)
    return out
